# revision 22
# baseline (speedup 1.0000x reference)
"""Conv2d (32,128,64,64) x (256,128,3,3) stride 1 pad 1 -> (32,256,64,64), f32.

Strategy: data-parallel over batch across 8 NeuronCores (4 images/core).
Per core, conv is computed as 9 PSUM-accumulated matmuls (one per kernel tap):
  out[o, y, x] += W[o, i, kh, kw] * xpad[i, y+kh, x+kw]
with contraction over i (=128, the partition dim). lhsT is the weight
transposed to [i, (tap, oc), o] — pre-transposed on the host and DMA'd in as
a contiguous extra input. rhs is read from a zero-padded [128, 66, 66] SBUF
image with a strided 2-D free access pattern. Each matmul covers 8 output
rows (N = 512) into one PSUM bank. Operands are bitcast to float32r (fp32
bits; the PE rounds to its 11-mantissa-bit fp32r format on read and runs
1 cycle/row instead of fp32's 4). Bias is fused into the PSUM->SBUF drain
on the vector engine. Input/output DMAs are chunked so the PE starts early
and the tail stays short.
"""

import numpy as np

B, CIN, H, W = 32, 128, 64, 64
COUT, KH, KW = 256, 3, 3
N_CORES = 8
B_LOC = B // N_CORES            # images per core
HP, WP = H + 2, W + 2           # padded image
ROWS = 8                        # output rows per matmul
NBLK = H // ROWS                # spatial blocks per image
NOC = COUT // 128               # output-channel chunks
NK = KH * KW
N_WARM = 10                     # PE warm-up matmuls at kernel start

_CACHE: dict = {}


def _build():
    import concourse.bacc as bacc
    import concourse.mybir as mybir
    import concourse.tile as tile

    f32 = mybir.dt.float32
    f32r = mybir.dt.float32r

    nc = bacc.Bacc(
        "TRN2",
        target_bir_lowering=False,
        debug=False,
        enable_asserts=False,
        num_devices=N_CORES,
    )
    x_d = nc.dram_tensor("input", (B_LOC, CIN, H, W), f32, kind="ExternalInput").ap()
    # host-pre-transposed weights: [i, oc, tap, o']
    wt_d = nc.dram_tensor("weights_t", (CIN, NOC, NK, 128), f32, kind="ExternalInput").ap()
    b_d = nc.dram_tensor("biases", (COUT,), f32, kind="ExternalInput").ap()
    y_d = nc.dram_tensor("out", (B_LOC, COUT, H, W), f32, kind="ExternalOutput").ap()

    with tile.TileContext(nc) as tc:
        with (
            tc.tile_pool(name="const", bufs=1) as const_pool,
            tc.tile_pool(name="xpad", bufs=4) as x_pool,
            tc.tile_pool(name="outsb", bufs=2) as out_pool,
            tc.tile_pool(name="psum", bufs=8, space="PSUM") as psum_pool,
        ):
            # PE warm-up: dummy matmuls on a zeroed scratch tile keep the PE
            # busy (and ramp its clock to full rate) while the first weight
            # and input DMAs land. The scratch PSUM result is never read.
            warm = const_pool.tile([128, 512], f32r)
            nc.vector.memset(warm[:, :].bitcast(f32), 0.0)
            wps = psum_pool.tile([128, 512], f32, tag="ps")
            for _ in range(N_WARM):
                nc.tensor.matmul(wps[:, :], warm[:, 0:128], warm[:, :],
                                 start=True, stop=True)

            wT = const_pool.tile([128, NOC, NK, 128], f32r)

            # biases (256,) -> [o', oc] so bias_t[:, oc] is per-partition
            bias_t = const_pool.tile([128, NOC], f32)

            def load_image(b, xp):
                # zero the halo ring; interior is fully overwritten by the DMAs
                nc.vector.memset(xp[:, 0, :].bitcast(f32), 0.0)
                nc.vector.memset(xp[:, HP - 1, :].bitcast(f32), 0.0)
                nc.vector.memset(xp[:, 1:H + 1, 0].bitcast(f32), 0.0)
                nc.vector.memset(xp[:, 1:H + 1, WP - 1].bitcast(f32), 0.0)

            def load_chunk(b, xp, ci):
                # raw-byte HWDGE DMAs; the PE rounds fp32r operands on read.
                # Chunked by row-groups so the first matmuls start early.
                r0 = ci * 16
                nc.sync.dma_start(
                    xp[:, r0 + 1:r0 + 17, 1:W + 1],
                    x_d[b, :, r0:r0 + 16, :].bitcast(f32r),
                )

            # Hand-ordered startup DMA queue: weights for oc=0, then the
            # first image's chunks, with oc=1 weights and bias slotted in
            # before the last chunk (each lands well before it is needed).
            nc.sync.dma_start(wT[:, 0], wt_d[:, 0].bitcast(f32r))
            xp0 = x_pool.tile([128, HP, WP], f32r)
            load_image(0, xp0)
            load_chunk(0, xp0, 0)
            load_chunk(0, xp0, 1)
            load_chunk(0, xp0, 2)
            nc.sync.dma_start(wT[:, 1], wt_d[:, 1].bitcast(f32r))
            nc.sync.dma_start(bias_t[:, :], b_d.rearrange("(a p) -> p a", p=128))
            load_chunk(0, xp0, 3)

            for b in range(B_LOC):
                if b == 0:
                    xp = xp0
                else:
                    xp = x_pool.tile([128, HP, WP], f32r)
                    load_image(b, xp)
                    for ci in range(4):
                        load_chunk(b, xp, ci)

                for oc in range(NOC):
                    # whole [128, 64, 64] output half staged in SBUF
                    ot = out_pool.tile([128, H * W], f32)
                    last_group = b == B_LOC - 1 and oc == NOC - 1
                    for s in range(NBLK):
                        ps = psum_pool.tile([128, ROWS * W], f32)
                        for kk in range(NK):
                            kh, kw = kk // KW, kk % KW
                            rhs = xp[:, s * ROWS + kh: s * ROWS + kh + ROWS, kw: kw + W]
                            nc.tensor.matmul(
                                ps[:, :],
                                wT[:, oc, kk, :],
                                rhs,
                                start=(kk == 0),
                                stop=(kk == NK - 1),
                            )
                        # flush drained blocks (contiguous in DRAM). The very
                        # last group flushes per-block to shorten the tail.
                        nc.vector.tensor_scalar_add(
                            ot[:, s * ROWS * W:(s + 1) * ROWS * W],
                            ps[:, :],
                            bias_t[:, oc:oc + 1],
                        )
                        if last_group:
                            nc.sync.dma_start(
                                y_d[b, oc * 128:(oc + 1) * 128, s * ROWS:(s + 1) * ROWS, :],
                                ot[:, s * ROWS * W:(s + 1) * ROWS * W],
                            )
                        elif s % 2 == 1:
                            nc.sync.dma_start(
                                y_d[b, oc * 128:(oc + 1) * 128, (s - 1) * ROWS:(s + 1) * ROWS, :],
                                ot[:, (s - 1) * ROWS * W:(s + 1) * ROWS * W],
                            )

    nc.compile()
    return nc


def get_nc():
    if "nc" not in _CACHE:
        _CACHE["nc"] = _build()
    return _CACHE["nc"]


def make_weights_t(weights):
    # wT[i, oc, kk, o'] = W[oc*128 + o', i, kh, kw], kk = kh*KW + kw
    w = np.ascontiguousarray(weights, dtype=np.float32)
    w = w.reshape(NOC, 128, CIN, NK)            # (oc, o', i, kk)
    w = w.transpose(2, 0, 3, 1)                 # (i, oc, kk, o')
    return np.ascontiguousarray(w)


def kernel(input, weights, biases):
    from concourse import bass_utils

    nc = get_nc()
    input = np.ascontiguousarray(input, dtype=np.float32)
    shards = input.reshape(N_CORES, B_LOC, CIN, H, W)
    wt = make_weights_t(weights)
    bs = np.ascontiguousarray(biases, dtype=np.float32)
    in_maps = [
        {"input": shards[c], "weights_t": wt, "biases": bs}
        for c in range(N_CORES)
    ]
    res = bass_utils.run_bass_kernel_spmd(nc, in_maps, core_ids=list(range(N_CORES)))
    return np.concatenate([res.results[c]["out"] for c in range(N_CORES)], axis=0)
